# revision 1
# baseline (speedup 1.0000x reference)
"""Additive (Bahdanau) attention kernel for Trainium2, SPMD over 8 NeuronCores.

score[b,l,k] = sum_a w3[a] * tanh(qp[b,l,a] + kp[b,k,a]);  masked softmax over k
  qp = Q @ W1^T, kp = K @ W2^T

Sharding: data-parallel over batch B=8 (one batch per core), weights replicated.

Algorithm: Fourier ridge decomposition. Since tanh saturates,
h(z) = tanh(z) - z/Z is odd and effectively smooth-periodic on [-Z, Z], so

  tanh(x+y) = (x+y)/Z + sum_m b_m sin(theta_m (x+y)),   theta_m = m pi / Z

with geometrically decaying b_m. Each sine mode splits by angle addition into
two separable products, so with F/V factor matrices over (l,a)/(k,a):

  score = sum_m [ (b_m w3 sin_m(qp)) @ cos_m(kp)^T
                 +(b_m w3 cos_m(qp)) @ sin_m(kp)^T ]
        + (w3 qp / Z) @ ones^T + (w3/Z rep) @ kp^T

-- all tensor-engine matmuls with contraction (a x 2M+2). The sin/cos factors
use exact range reduction: u = x*(theta/2pi) + C with C = 1.5*2^23 rounds to
the nearest integer in the add itself; w = u - C = round(t); the residual
theta*x - 2pi*w lands in [-pi, pi] where ACT's Sin is exact. The subtract
runs on the PE (scaled-identity matmuls into PSUM, ACT reads PSUM) for some
modes and on DVE (scalar_tensor_tensor) for the rest, to balance engines.
Arguments beyond [-Z, Z] wrap onto the periodic extension, which still
matches tanh to ~1e-4 out to |x+y| ~ 2Z - 3.5 because tanh is flat there.
"""

import sys

import numpy as np

if "/opt/trn_rl_repo" not in sys.path:
    sys.path.insert(0, "/opt/trn_rl_repo")

B, LQ, LK, D, A = 8, 256, 256, 512, 256
N_CORES = 8

ZP = 5.5       # half-period of the Fourier expansion
M = 6          # number of sine modes
CMAGIC = float(1.5 * 2 ** 23)   # fp32 round-to-nearest-integer magic constant
N_DVE_FRAC = 7  # of the 2M (mode,fn) units, this many subtract on DVE

_cached_nc = None


def _fourier_coeffs(mmax, zp, n=1 << 16):
    z = (np.arange(n) + 0.5) / n * 2 * zp - zp
    h = np.tanh(z) - z / zp
    b = np.zeros(mmax + 1)
    for m in range(1, mmax + 1):
        b[m] = (1.0 / zp) * np.trapezoid(h * np.sin(m * np.pi * z / zp), z)
    return b


def _build():
    from contextlib import ExitStack

    import concourse.mybir as mybir
    from concourse import tile
    from concourse.bacc import Bacc

    FP = mybir.dt.float32
    BF = mybir.dt.bfloat16
    I32 = mybir.dt.int32
    Act = mybir.ActivationFunctionType
    Alu = mybir.AluOpType

    bm = _fourier_coeffs(M, ZP)
    thetas = [m * np.pi / ZP for m in range(1, M + 1)]

    nc = Bacc()
    Qd = nc.declare_dram_parameter("QT", [D, LQ], FP, isOutput=False)
    Kd = nc.declare_dram_parameter("KT", [D, LK], FP, isOutput=False)
    Md = nc.declare_dram_parameter("mask", [LQ, LK], I32, isOutput=False)
    W1d = nc.declare_dram_parameter("W1T", [D, A], FP, isOutput=False)
    W2d = nc.declare_dram_parameter("W2T", [D, A], FP, isOutput=False)
    w3d = nc.declare_dram_parameter("w3", [A], FP, isOutput=False)
    Id = nc.declare_dram_parameter("ident", [128, 128], FP, isOutput=False)
    Od = nc.declare_dram_parameter("out", [LQ, LK], FP, isOutput=True)

    with tile.TileContext(nc) as tc:
        with ExitStack() as ctx:
            const = ctx.enter_context(tc.tile_pool(name="const", bufs=1))
            load = ctx.enter_context(tc.tile_pool(name="load", bufs=1))
            trans = ctx.enter_context(tc.tile_pool(name="trans", bufs=1))
            proj = ctx.enter_context(tc.tile_pool(name="proj", bufs=1))
            fact = ctx.enter_context(tc.tile_pool(name="fact", bufs=1))
            uw = ctx.enter_context(tc.tile_pool(name="uw", bufs=4))
            smx = ctx.enter_context(tc.tile_pool(name="smx", bufs=2))
            pp = ctx.enter_context(tc.tile_pool(name="pp", bufs=2, space="PSUM"))
            pfr = ctx.enter_context(tc.tile_pool(name="pfr", bufs=4, space="PSUM"))
            psc = ctx.enter_context(tc.tile_pool(name="psc", bufs=1, space="PSUM"))

            # input DMAs first -- nothing depends on program position,
            # and the transposed layouts load directly (host pre-transposes)
            qT = trans.tile([128, 4, 256], FP)
            kT = trans.tile([128, 4, 256], FP)
            w1T = trans.tile([128, 4, 256], FP)
            w2T = trans.tile([128, 4, 256], FP)
            dma_engines = [nc.sync, nc.scalar, nc.gpsimd, nc.sync]
            for si, (dst, dr) in enumerate(
                ((qT, Qd), (w1T, W1d), (kT, Kd), (w2T, W2d))
            ):
                dma_engines[si].dma_start(
                    dst[:], dr.rearrange("(db p) x -> p db x", p=128)
                )
            mi = load.tile([128, 2, 256], I32)
            nc.sync.dma_start(mi[:], Md.rearrange("(i p) k -> p i k", p=128))
            w3_sb = const.tile([128, 2], FP)
            nc.sync.dma_start(w3_sb[:], w3d.rearrange("(j p) -> p j", p=128))
            ident = const.tile([128, 128], FP)
            nc.sync.dma_start(ident[:], Id[:])

            # scaled identities for the PE frac path
            iths = []
            for m in range(M):
                ith = const.tile([128, 128], FP, name=f"ith{m}")
                nc.vector.tensor_scalar_mul(ith[:], ident[:], float(thetas[m]))
                iths.append(ith)
            ineg = const.tile([128, 128], FP)
            nc.vector.tensor_scalar_mul(ineg[:], ident[:], float(-2 * np.pi))

            bias_hp = const.tile([128, 1], FP)
            nc.vector.memset(bias_hp[:], float(np.pi / 2))
            bias_cm = const.tile([128, 1], FP)
            nc.vector.memset(bias_cm[:], CMAGIC)
            ones_k = const.tile([128, 256], BF)
            nc.vector.memset(ones_k[:], 1.0)

            # w3 * b_m columns for ACT-side folds (scale AP per partition)
            w3b = const.tile([128, M, 2], FP)
            for m in range(M):
                nc.vector.tensor_scalar_mul(
                    w3b[:, m, :], w3_sb[:], float(bm[m + 1])
                )
            # w3/Z replicated across 128 columns, per a-tile (k-linear lhsT)
            w3z = const.tile([128, 2, 128], BF)
            for at in range(2):
                nc.vector.tensor_copy(
                    w3z[:, at, :], w3_sb[:, at:at + 1].broadcast_to([128, 128])
                )
            nc.vector.tensor_scalar_mul(w3z[:], w3z[:], float(1.0 / ZP))

            # projections -> qkpT [a 128][4][256]: [0:2]=qp, [2:4]=kp
            qkpT = proj.tile([128, 4, 256], FP)

            def project(xT, wT, si):
                for at in range(2):
                    pj = pp.tile([128, 256], FP, name="ppool")
                    for db in range(4):
                        nc.tensor.matmul(
                            pj[:],
                            wT[:, db, at * 128:(at + 1) * 128],
                            xT[:, db, :],
                            start=(db == 0),
                            stop=(db == 3),
                        )
                    nc.scalar.copy(qkpT[:, 2 * si + at, :], pj[:])

            project(qT, w1T, 0)
            project(kT, w2T, 1)

            # bf16 kp copy for the k-side linear matmul term
            kp_bf = proj.tile([128, 2, 256], BF)
            nc.vector.tensor_copy(kp_bf[:], qkpT[:, 2:4, :])

            # mask -> additive bias (gpsimd; off the hot engines)
            mb = proj.tile([128, 2, 256], FP)
            nc.gpsimd.tensor_copy(mb[:], mi[:])
            nc.gpsimd.tensor_scalar(
                mb[:], mb[:], 1.0e15, -1.0e15, op0=Alu.mult, op1=Alu.add
            )

            # ---- mode-pipelined factor evaluation + score matmuls ---------
            # Per mode: DVE chains -> sin/cos eval -> q-side folds -> this
            # mode's 8 score matmuls. PE-path frac units go to the EARLY
            # modes so the in-order PE stream never waits on late factors.
            sc0 = psc.tile([128, 256], FP)
            sc1 = psc.tile([128, 256], FP)
            scores = [sc0[:], sc1[:]]
            n_per_lc = (2 * M + 2) * 2
            cnt = [0, 0]

            # linear-term q-side factor first (cheap, unblocks nothing)
            flin = fact.tile([128, 2, 256], BF)
            for at in range(2):
                nc.vector.tensor_scalar(
                    flin[:, at, :], qkpT[:, at, :],
                    w3_sb[:, at:at + 1], float(1.0 / ZP),
                    op0=Alu.mult, op1=Alu.mult,
                )

            def score_mm(lc, lhsT, rhs):
                nc.tensor.matmul(
                    scores[lc], lhsT, rhs,
                    start=(cnt[lc] == 0), stop=(cnt[lc] == n_per_lc - 1),
                )
                cnt[lc] += 1

            # linear terms open the accumulation
            for at in range(2):
                for lc in range(2):
                    score_mm(lc, flin[:, at, lc * 128:(lc + 1) * 128],
                             ones_k[:])
                    score_mm(lc, w3z[:, at, :], kp_bf[:, at, :])

            n_pe_frac = 2 * M - N_DVE_FRAC
            unit = 0
            for m in range(M):
                t2p = float(thetas[m] / (2 * np.pi))
                fpair = []
                for fn in range(2):
                    if fn == 0:
                        u = uw.tile([128, 4, 256], FP, name="u")
                        if m % 2 == 1:
                            nc.scalar.activation(
                                u[:], qkpT[:], Act.Identity,
                                bias=bias_cm[:, 0:1], scale=t2p,
                            )
                        else:
                            nc.vector.tensor_scalar(
                                u[:], qkpT[:], t2p, CMAGIC,
                                op0=Alu.mult, op1=Alu.add,
                            )
                    else:
                        tc_ = uw.tile([128, 4, 256], FP, name="tc")
                        nc.vector.tensor_scalar(
                            tc_[:], qkpT[:], t2p, 0.25,
                            op0=Alu.mult, op1=Alu.add,
                        )
                        u = None
                    fac = fact.tile([128, 4, 256], BF, name=f"fac{m}{fn}")
                    if unit >= n_pe_frac:
                        w_ = uw.tile([128, 4, 256], FP, name="w")
                        if u is None:
                            wr = uw.tile([128, 4, 256], FP, name="wr")
                            nc.vector.tensor_scalar(
                                wr[:], tc_[:], CMAGIC, -CMAGIC,
                                op0=Alu.add, op1=Alu.add,
                            )
                            nc.vector.tensor_scalar(
                                w_[:], wr[:], float(2 * np.pi), None,
                                op0=Alu.mult,
                            )
                        else:
                            nc.vector.tensor_scalar(
                                w_[:], u[:], -CMAGIC, float(2 * np.pi),
                                op0=Alu.add, op1=Alu.mult,
                            )
                        fr = uw.tile([128, 4, 256], FP, name="fr")
                        nc.vector.scalar_tensor_tensor(
                            fr[:], qkpT[:], float(thetas[m]), w_[:],
                            op0=Alu.mult, op1=Alu.subtract,
                        )
                        nc.scalar.activation(
                            fac[:], fr[:], Act.Sin,
                            bias=(bias_hp[:, 0:1] if fn else 0.0),
                        )
                    else:
                        w_ = uw.tile([128, 4, 256], FP, name="w")
                        if u is None:
                            nc.vector.tensor_scalar(
                                w_[:], tc_[:], CMAGIC, -CMAGIC,
                                op0=Alu.add, op1=Alu.add,
                            )
                        else:
                            nc.vector.tensor_scalar(
                                w_[:], u[:], -CMAGIC, None, op0=Alu.add,
                            )
                        for half in range(2):
                            sl = slice(2 * half, 2 * half + 2)
                            frh = pfr.tile([128, 2, 256], FP, name="pfr")
                            nc.tensor.matmul(
                                frh[:], iths[m],
                                qkpT[:, sl, :].rearrange("p i k -> p (i k)"),
                                start=True, stop=False,
                            )
                            nc.tensor.matmul(
                                frh[:], ineg[:],
                                w_[:, sl, :].rearrange("p i k -> p (i k)"),
                                start=False, stop=True,
                            )
                            nc.scalar.activation(
                                fac[:, sl, :], frh[:], Act.Sin,
                                bias=(bias_hp[:, 0:1] if fn else 0.0),
                            )
                    fpair.append(fac)
                    unit += 1
                # q-side folds for this mode (split across DVE and ACT)
                for fn in range(2):
                    f = fpair[fn]
                    for at in range(2):
                        if (fn + at) % 2 == 0:
                            nc.vector.tensor_scalar(
                                f[:, at, :], f[:, at, :],
                                w3_sb[:, at:at + 1], float(bm[m + 1]),
                                op0=Alu.mult, op1=Alu.mult,
                            )
                        else:
                            nc.scalar.activation(
                                f[:, at, :], f[:, at, :], Act.Identity,
                                bias=0.0, scale=w3b[:, m, at:at + 1],
                            )
                # this mode's score matmuls; last mode closes sc0 first
                if m == M - 1:
                    for lc in range(2):
                        for fq_fn, vk_fn in ((0, 1), (1, 0)):
                            for at in range(2):
                                score_mm(
                                    lc,
                                    fpair[fq_fn][:, at,
                                                 lc * 128:(lc + 1) * 128],
                                    fpair[vk_fn][:, 2 + at, :],
                                )
                else:
                    for fq_fn, vk_fn in ((0, 1), (1, 0)):
                        for at in range(2):
                            for lc in range(2):
                                score_mm(
                                    lc,
                                    fpair[fq_fn][:, at,
                                                 lc * 128:(lc + 1) * 128],
                                    fpair[vk_fn][:, 2 + at, :],
                                )

            # ---- masked softmax over k ------------------------------------
            for lb in range(2):
                masked = smx.tile([128, 256], FP)
                nc.vector.tensor_add(masked[:], scores[lb], mb[:, lb, :])
                e = smx.tile([128, 256], FP)
                sums = smx.tile([128, 1], FP)
                nc.scalar.activation(
                    e[:], masked[:], Act.Exp,
                    bias=0.0, scale=1.0, accum_out=sums[:],
                )
                recip = smx.tile([128, 1], FP)
                nc.vector.reciprocal(recip[:], sums[:])
                outt = smx.tile([128, 256], FP)
                nc.vector.tensor_scalar_mul(outt[:], e[:], recip[:])
                nc.sync.dma_start(Od[lb * 128:(lb + 1) * 128, :], outt[:])

    nc.compile()
    return nc


def _get_nc():
    global _cached_nc
    if _cached_nc is None:
        _cached_nc = _build()
    return _cached_nc


def _make_in_maps(inputs):
    Q = np.ascontiguousarray(
        np.asarray(inputs["Q"], dtype=np.float32).reshape(B, LQ, D)
    )
    K = np.ascontiguousarray(
        np.asarray(inputs["K"], dtype=np.float32).reshape(B, LK, D)
    )
    mask = np.ascontiguousarray(np.asarray(inputs["mask"], dtype=np.int32))
    W1 = np.ascontiguousarray(np.asarray(inputs["W1"], dtype=np.float32))
    W2 = np.ascontiguousarray(np.asarray(inputs["W2"], dtype=np.float32))
    w3 = np.ascontiguousarray(np.asarray(inputs["w3"], dtype=np.float32))
    W1T = np.ascontiguousarray(W1.T)
    W2T = np.ascontiguousarray(W2.T)
    ident = np.eye(128, dtype=np.float32)
    return [
        dict(
            QT=np.ascontiguousarray(Q[i].T),
            KT=np.ascontiguousarray(K[i].T),
            mask=mask[i], W1T=W1T, W2T=W2T, w3=w3, ident=ident,
        )
        for i in range(N_CORES)
    ]


def _run(inputs, trace=False, tmpdir=None):
    from concourse.bass_utils import run_bass_kernel_spmd

    nc = _get_nc()
    in_maps = _make_in_maps(inputs)
    res = run_bass_kernel_spmd(
        nc, in_maps, list(range(N_CORES)), trace=trace, tmpdir=tmpdir
    )
    out = np.stack([res.results[i]["out"] for i in range(N_CORES)], axis=0)
    return out, res


def kernel(**inputs) -> np.ndarray:
    out, _ = _run(inputs, trace=False)
    return out



# revision 6
# speedup vs baseline: 1.6645x; 1.6645x over previous
"""Additive (Bahdanau) attention for Trainium2, SPMD over 8 NeuronCores.

score[b,l,k] = sum_a w3[a] * tanh(qp[b,l,a] + kp[b,k,a]);  masked softmax over k
  qp = Q @ W1^T, kp = K @ W2^T

Sharding: data-parallel over batch B=8 (one batch per core), weights replicated.

Algorithm: 3-mode harmonic sine-ridge fit of tanh, theta tuned on the
empirical z = qp+kp distribution and validated against the measured ACT-Sin
error curve:

  tanh(z) ~= c_lin*z + b1 sin(t z) + b2 sin(2 t z) + b3 sin(3 t z)

Each sine mode splits by angle addition into separable products over the
a-axis -> bf16 tensor-engine matmuls with contraction A. Mode-1 sin/cos come
straight from ACT Sin reading the projection PSUM. Higher modes avoid the
slow 1x scalar_tensor_tensor path by expanding the q-side in *monomials* of
mode-1 factors (U3=s1c1, U4=c1^2, V5=c1U3, V6=c1U4 -- pure 2x tensor_tensor)
and keeping the few affine fix-ups on the k side:

  sin2(q+k): 2 U3q c2k + 2 U4q s2k - s2k        (rank-1 in k -> extra matmul)
  sin3(q+k): (4V5-s1)q c3k + (4V6-3c1)q s3k     (s1/c1 lhsT reused)

w3 folds onto the q side as plain tensor_tensor against host-replicated
broadcast tensors (AP-scalar tensor_scalar is 10x slow). The q-side linear
rank-1 term cancels in the softmax; the k-side linear term is one matmul
against a host-built w3*c_lin lhsT. Softmax: additive -100 mask bias, ACT Exp
with accumulated row sums, normalize via ACT Identity with per-partition
reciprocal scale.
"""

import sys

import numpy as np

if "/opt/trn_rl_repo" not in sys.path:
    sys.path.insert(0, "/opt/trn_rl_repo")

import ml_dtypes

B, LQ, LK, D, A = 8, 256, 256, 512, 256
N_CORES = 8

THETA = 0.78
C_LIN = 0.2467
B1, B2, B3 = 0.50004, 0.12708, 0.04244

_cached_nc = None


def _build():
    from contextlib import ExitStack

    import concourse.mybir as mybir
    from concourse import tile
    from concourse.bacc import Bacc

    FP = mybir.dt.float32
    BF = mybir.dt.bfloat16
    Act = mybir.ActivationFunctionType
    Alu = mybir.AluOpType

    nc = Bacc()
    KBd = nc.declare_dram_parameter("blob_k", [128, 2048], BF, isOutput=False)
    QBd = nc.declare_dram_parameter("blob_q", [128, 2048], BF, isOutput=False)
    # w3 blob: w3*b1 bcast [2,2,256] | w3 bcast [4,2,256] | w3*c_lin rep [2,128]
    WBd = nc.declare_dram_parameter("blob_w", [128, 3328], BF, isOutput=False)
    Md = nc.declare_dram_parameter("maskb", [128, 512], BF, isOutput=False)
    Od = nc.declare_dram_parameter("out", [128, 512], FP, isOutput=True)

    with tile.TileContext(nc) as tc:
        with ExitStack() as ctx:
            const = ctx.enter_context(tc.tile_pool(name="const", bufs=1))
            inp = ctx.enter_context(tc.tile_pool(name="inp", bufs=1))
            fk = ctx.enter_context(tc.tile_pool(name="fk", bufs=1))
            fq = ctx.enter_context(tc.tile_pool(name="fq", bufs=1))
            tl = ctx.enter_context(tc.tile_pool(name="tl", bufs=1))
            ppk = ctx.enter_context(tc.tile_pool(name="ppk", bufs=1, space="PSUM"))
            ppq = ctx.enter_context(tc.tile_pool(name="ppq", bufs=1, space="PSUM"))
            ps = ctx.enter_context(tc.tile_pool(name="ps", bufs=1, space="PSUM"))

            # ---- input DMAs (HWDGE; complete during the startup barrier) --
            kin = inp.tile([128, 2, 4, 256], BF)     # [kt | w2]
            nc.sync.dma_start(kin[:], KBd.rearrange("p (i db x) -> p i db x",
                                                    i=2, db=4))
            qin = inp.tile([128, 2, 4, 256], BF)     # [qt | w1]
            nc.sync.dma_start(qin[:], QBd.rearrange("p (i db x) -> p i db x",
                                                    i=2, db=4))
            win = inp.tile([128, 3328], BF)
            nc.scalar.dma_start(win[:], WBd[:])
            maskb = inp.tile([128, 2, 256], BF)
            nc.scalar.dma_start(maskb[:], Md.rearrange("p (i k) -> p i k", i=2))

            kt, w2 = kin[:, 0], kin[:, 1]
            qt, w1 = qin[:, 0], qin[:, 1]
            w3b1bc = win[:, 0:1024].rearrange("p (u i k) -> p u i k", u=2, i=2)
            w3bc = win[:, 1024:3072].rearrange("p (u i k) -> p u i k", u=4, i=2)
            w3rep = win[:, 3072:3328].rearrange("p (i k) -> p i k", i=2)

            junk = const.tile([128, 8], BF)
            nc.gpsimd.memset(junk[:], 0.125)
            bias_hp = const.tile([128, 1], FP)
            nc.gpsimd.memset(bias_hp[:], float(np.pi / 2))
            dummy = const.tile([128, 8], FP)
            # first ACT op: forces the sin table load at t~0
            nc.scalar.activation(dummy[:], junk[:], Act.Sin, bias=0.0)

            # ---- projections: kp = W2 K^T, qp = W1 Q^T (PSUM fp32) -------
            PK = ppk.tile([128, 2, 256], FP)
            for at in range(2):
                for db in range(4):
                    nc.tensor.matmul(PK[:, at, :],
                                     w2[:, db, at * 128:(at + 1) * 128],
                                     kt[:, db, :],
                                     start=(db == 0), stop=(db == 3))
            PQ = ppq.tile([128, 2, 256], FP)
            for at in range(2):
                for db in range(4):
                    nc.tensor.matmul(PQ[:, at, :],
                                     w1[:, db, at * 128:(at + 1) * 128],
                                     qt[:, db, :],
                                     start=(db == 0), stop=(db == 3))

            # ---- mode-1 factors on ACT ------------------------------------
            S1k = fk.tile([128, 2, 256], BF)
            C1k = fk.tile([128, 2, 256], BF)
            nc.scalar.activation(S1k[:], PK[:], Act.Sin, bias=0.0, scale=THETA)
            nc.scalar.activation(C1k[:], PK[:], Act.Sin,
                                 bias=bias_hp[:, 0:1], scale=THETA)
            # FQ units: 0=s1,1=c1,2=U3=s1c1,3=U4=c1^2,4=V5=c1U3,5=V6=c1U4
            FQ = fq.tile([128, 6, 2, 256], BF)
            GQ = fq.tile([128, 6, 2, 256], BF)
            kpbf = fk.tile([128, 2, 256], BF)
            nc.scalar.activation(kpbf[:], PK[:], Act.Identity, bias=0.0)
            nc.scalar.activation(FQ[:, 0], PQ[:], Act.Sin, bias=0.0,
                                 scale=THETA)
            nc.scalar.activation(FQ[:, 1], PQ[:], Act.Sin,
                                 bias=bias_hp[:, 0:1], scale=THETA)

            # ---- DVE: monomials, k-side mode tensors, folds ---------------
            def tt(out, a, b, op=Alu.mult):
                nc.vector.tensor_tensor(out, a, b, op=op)

            def ts(out, in0, s1, s2=None, op0=Alu.mult, op1=Alu.add):
                if s2 is None:
                    nc.vector.tensor_scalar(out, in0, float(s1), None, op0=op0)
                else:
                    nc.vector.tensor_scalar(out, in0, float(s1), float(s2),
                                            op0=op0, op1=op1)

            # k mode-2 (ready right after k sins)
            X2 = fk.tile([128, 2, 256], BF)
            Y2 = fk.tile([128, 2, 256], BF)
            tt(X2[:], S1k[:], C1k[:])
            tt(Y2[:], C1k[:], C1k[:])
            C2s = fk.tile([128, 2, 256], BF)
            S2s = fk.tile([128, 2, 256], BF)
            S2r = fk.tile([128, 2, 256], BF)
            ts(C2s[:], Y2[:], 4 * B2, -2 * B2)
            ts(S2s[:], X2[:], 4 * B2)
            ts(S2r[:], X2[:], -2 * B2 / C_LIN)
            # q monomials + mode-1 folds
            tt(GQ[:, 0:2], FQ[:, 0:2], w3b1bc[:, 0:2])
            tt(FQ[:, 2], FQ[:, 0], FQ[:, 1])
            tt(FQ[:, 3], FQ[:, 1], FQ[:, 1])
            tt(FQ[:, 4], FQ[:, 1], FQ[:, 2])
            tt(FQ[:, 5], FQ[:, 1], FQ[:, 3])
            tt(GQ[:, 2:6], FQ[:, 2:6], w3bc[:])
            # k mode-3
            T3 = fk.tile([128, 2, 256], BF)
            U3k = fk.tile([128, 2, 256], BF)
            tt(T3[:], C1k[:], X2[:])
            tt(U3k[:], C1k[:], Y2[:])
            s3B = fk.tile([128, 2, 256], BF)
            c3B = fk.tile([128, 2, 256], BF)
            nc.vector.scalar_tensor_tensor(s3B[:], S1k[:], -0.25, T3[:],
                                           op0=Alu.mult, op1=Alu.add)
            nc.vector.scalar_tensor_tensor(c3B[:], C1k[:], -0.75, U3k[:],
                                           op0=Alu.mult, op1=Alu.add)
            S3s = fk.tile([128, 2, 256], BF)
            S3r = fk.tile([128, 2, 256], BF)
            C3s = fk.tile([128, 2, 256], BF)
            C3r = fk.tile([128, 2, 256], BF)
            ts(S3s[:], s3B[:], 16 * B3)
            ts(S3r[:], s3B[:], -12 * B3 / B1)
            ts(C3s[:], c3B[:], 16 * B3)
            ts(C3r[:], c3B[:], -4 * B3 / B1)

            # ---- score matmuls into two PSUM l-tiles ----------------------
            S0 = ps.tile([128, 256], FP)
            S1 = ps.tile([128, 256], FP)
            Sl = [S0, S1]
            cnt = [0, 0]
            n_mm = 20

            def score_mm(lt, lhsT, rhs):
                nc.tensor.matmul(Sl[lt][:], lhsT, rhs, start=(cnt[lt] == 0),
                                 stop=(cnt[lt] == n_mm - 1))
                cnt[lt] += 1

            # (q-unit lhsT, k rhs) pairs in readiness order
            plan = [("rep", kpbf),                 # k-linear
                    (0, C1k), (1, S1k),            # mode 1
                    (2, C2s), (3, S2s),            # mode 2
                    ("rep", S2r),                  # mode-2 rank-1
                    (4, C3s), (5, S3s),            # mode 3
                    (0, C3r), (1, S3r)]            # mode-3 s1/c1 partners
            for u, krhs in plan:
                for at in range(2):
                    for lt in range(2):
                        lhsT = (w3rep[:, at, :] if u == "rep"
                                else GQ[:, u, at, lt * 128:(lt + 1) * 128])
                        score_mm(lt, lhsT, krhs[:, at, :])

            # ---- masked softmax over k ------------------------------------
            for lt in range(2):
                ms = tl.tile([128, 256], FP, name=f"ms{lt}")
                tt(ms[:], Sl[lt][:], maskb[:, lt, :], op=Alu.add)
                e = tl.tile([128, 256], BF, name=f"e{lt}")
                sums = tl.tile([128, 1], FP, name=f"sums{lt}")
                nc.scalar.activation(e[:], ms[:], Act.Exp, bias=0.0,
                                     accum_out=sums[:])
                recip = tl.tile([128, 1], FP, name=f"recip{lt}")
                nc.vector.reciprocal(recip[:], sums[:])
                outt = tl.tile([128, 256], FP, name=f"outt{lt}")
                nc.scalar.activation(outt[:], e[:], Act.Identity, bias=0.0,
                                     scale=recip[:, 0:1])
                nc.sync.dma_start(Od[:, lt * 256:(lt + 1) * 256], outt[:])

    nc.compile()
    return nc


def _get_nc():
    global _cached_nc
    if _cached_nc is None:
        _cached_nc = _build()
    return _cached_nc


def _pack_T(x):
    """[rows, D=512] -> bf16 [128, 4*rows] laid out as (d%128, d//128, row)."""
    xT = np.ascontiguousarray(x.T)  # [D, rows]
    r = xT.reshape(4, 128, -1).transpose(1, 0, 2)  # [128, 4, rows]
    return np.ascontiguousarray(r.reshape(128, -1).astype(ml_dtypes.bfloat16))


def _make_in_maps(inputs):
    Q = np.asarray(inputs["Q"], dtype=np.float32).reshape(B, LQ, D)
    K = np.asarray(inputs["K"], dtype=np.float32).reshape(B, LK, D)
    mask = np.asarray(inputs["mask"])
    W1 = np.asarray(inputs["W1"], dtype=np.float32)
    W2 = np.asarray(inputs["W2"], dtype=np.float32)
    w3 = np.asarray(inputs["w3"], dtype=np.float32)

    w1p = _pack_T(W1)
    w2p = _pack_T(W2)
    w3t = w3.reshape(2, 128).T.astype(np.float32)          # [128 p, 2 at]
    bc = lambda x, u: np.repeat(np.repeat(x.T[None, :, :, None], u, axis=0),
                                256, axis=3).transpose(2, 0, 1, 3)
    # w3b1bc: [128, 2u, 2at, 256]; w3bc: [128, 4u, 2at, 256]
    w3b1bc = bc(w3t * B1, 2).reshape(128, -1).astype(ml_dtypes.bfloat16)
    w3bc = bc(w3t, 4).reshape(128, -1).astype(ml_dtypes.bfloat16)
    w3rep = np.repeat((w3t * C_LIN)[:, :, None], 128,
                      axis=2).reshape(128, -1).astype(ml_dtypes.bfloat16)
    blob_w = np.ascontiguousarray(
        np.concatenate([w3b1bc, w3bc, w3rep], axis=1))      # [128, 3328]

    maps = []
    for c in range(N_CORES):
        blob_k = np.concatenate([_pack_T(K[c]), w2p], axis=1)
        blob_q = np.concatenate([_pack_T(Q[c]), w1p], axis=1)
        mb = np.where(mask[c] == 0, -100.0, 0.0).astype(ml_dtypes.bfloat16)
        mb = np.ascontiguousarray(
            mb.reshape(2, 128, 256).transpose(1, 0, 2).reshape(128, 512))
        maps.append(dict(blob_k=np.ascontiguousarray(blob_k),
                         blob_q=np.ascontiguousarray(blob_q),
                         blob_w=blob_w,
                         maskb=mb))
    return maps


def _run(inputs, trace=False, tmpdir=None):
    from concourse.bass_utils import run_bass_kernel_spmd

    nc = _get_nc()
    in_maps = _make_in_maps(inputs)
    res = run_bass_kernel_spmd(
        nc, in_maps, list(range(N_CORES)), trace=trace, tmpdir=tmpdir
    )
    out = np.empty((B, LQ, LK), np.float32)
    for c in range(N_CORES):
        o = np.asarray(res.results[c]["out"], dtype=np.float32)  # [128, 512]
        out[c] = o.reshape(128, 2, 256).transpose(1, 0, 2).reshape(256, 256)
    return out, res


def kernel(**inputs) -> np.ndarray:
    out, _ = _run(inputs, trace=False)
    return out


# revision 7
# speedup vs baseline: 1.8566x; 1.1154x over previous
"""Additive (Bahdanau) attention for Trainium2, SPMD over 8 NeuronCores.

score[b,l,k] = sum_a w3[a] * tanh(qp[b,l,a] + kp[b,k,a]);  masked softmax over k
  qp = Q @ W1^T, kp = K @ W2^T

Sharding: data-parallel over batch B=8 (one batch per core), weights replicated.

Algorithm: 3-mode harmonic sine-ridge fit of tanh, theta tuned on the
empirical z = qp+kp distribution and validated against the measured ACT-Sin
error curve:

  tanh(z) ~= c_lin*z + b1 sin(t z) + b2 sin(2 t z) + b3 sin(3 t z)

Each sine mode splits by angle addition into separable products over the
a-axis -> bf16 tensor-engine matmuls with contraction A. Mode-1 sin/cos come
straight from ACT Sin reading the projection PSUM. Higher modes avoid the
slow 1x scalar_tensor_tensor path by expanding the q-side in *monomials* of
mode-1 factors (U3=s1c1, U4=c1^2, V5=c1U3, V6=c1U4 -- pure 2x tensor_tensor)
and keeping the few affine fix-ups on the k side:

  sin2(q+k): 2 U3q c2k + 2 U4q s2k - s2k        (rank-1 in k -> extra matmul)
  sin3(q+k): (4V5-s1)q c3k + (4V6-3c1)q s3k     (s1/c1 lhsT reused)

w3 folds onto the q side as plain tensor_tensor against host-replicated
broadcast tensors (AP-scalar tensor_scalar is 10x slow). The q-side linear
rank-1 term cancels in the softmax; the k-side linear term is one matmul
against a host-built w3*c_lin lhsT. Softmax: additive -100 mask bias, ACT Exp
with accumulated row sums, normalize via ACT Identity with per-partition
reciprocal scale.
"""

import sys

import numpy as np

if "/opt/trn_rl_repo" not in sys.path:
    sys.path.insert(0, "/opt/trn_rl_repo")

import ml_dtypes

B, LQ, LK, D, A = 8, 256, 256, 512, 256
N_CORES = 8

THETA = 0.78
C_LIN = 0.2467
B1, B2, B3 = 0.50004, 0.12708, 0.04244

_cached_nc = None


def _build():
    from contextlib import ExitStack

    import concourse.mybir as mybir
    from concourse import tile
    from concourse.bacc import Bacc

    FP = mybir.dt.float32
    BF = mybir.dt.bfloat16
    Act = mybir.ActivationFunctionType
    Alu = mybir.AluOpType

    nc = Bacc()
    KBd = nc.declare_dram_parameter("blob_k", [128, 2048], BF, isOutput=False)
    QBd = nc.declare_dram_parameter("blob_q", [128, 2048], BF, isOutput=False)
    # w3 blob: w3*b1 bcast [2,256] | w3 bcast [2,256] | w3*c_lin rep [2,128]
    WBd = nc.declare_dram_parameter("blob_w", [128, 1280], BF, isOutput=False)
    Md = nc.declare_dram_parameter("maskb", [128, 512], BF, isOutput=False)
    Od = nc.declare_dram_parameter("out", [128, 512], FP, isOutput=True)

    with tile.TileContext(nc) as tc:
        with ExitStack() as ctx:
            const = ctx.enter_context(tc.tile_pool(name="const", bufs=1))
            inp = ctx.enter_context(tc.tile_pool(name="inp", bufs=1))
            fk = ctx.enter_context(tc.tile_pool(name="fk", bufs=1))
            fq = ctx.enter_context(tc.tile_pool(name="fq", bufs=1))
            tl = ctx.enter_context(tc.tile_pool(name="tl", bufs=1))
            pw = ctx.enter_context(tc.tile_pool(name="pw", bufs=1, space="PSUM"))
            ctx_pools = {"pw": pw}
            ppk = ctx.enter_context(tc.tile_pool(name="ppk", bufs=1, space="PSUM"))
            ppq = ctx.enter_context(tc.tile_pool(name="ppq", bufs=1, space="PSUM"))
            ps = ctx.enter_context(tc.tile_pool(name="ps", bufs=1, space="PSUM"))

            # ---- input DMAs (HWDGE; complete during the startup barrier) --
            kin = inp.tile([128, 2, 4, 256], BF)     # [kt | w2]
            nc.sync.dma_start(kin[:], KBd.rearrange("p (i db x) -> p i db x",
                                                    i=2, db=4))
            qin = inp.tile([128, 2, 4, 256], BF)     # [qt | w1]
            nc.sync.dma_start(qin[:], QBd.rearrange("p (i db x) -> p i db x",
                                                    i=2, db=4))
            win = inp.tile([128, 1280], BF)
            nc.scalar.dma_start(win[:], WBd[:])
            maskb = inp.tile([128, 2, 256], BF)
            nc.scalar.dma_start(maskb[:], Md.rearrange("p (i k) -> p i k", i=2))

            kt, w2 = kin[:, 0], kin[:, 1]
            qt, w1 = qin[:, 0], qin[:, 1]
            w3b1c = win[:, 0:512].rearrange("p (i k) -> p i k", i=2)
            w3c = win[:, 512:1024].rearrange("p (i k) -> p i k", i=2)
            w3rep = win[:, 1024:1280].rearrange("p (i k) -> p i k", i=2)

            junk = const.tile([128, 8], BF)
            nc.gpsimd.memset(junk[:], 0.125)
            bias_hp = const.tile([128, 1], FP)
            nc.gpsimd.memset(bias_hp[:], float(np.pi / 2))
            dummy = const.tile([128, 8], FP)
            # first ACT op: forces the sin table load at t~0
            nc.scalar.activation(dummy[:], junk[:], Act.Sin, bias=0.0)
            wjunk = const.tile([128, 384], BF)
            nc.vector.memset(wjunk[:], 0.125)
            pwarm = ctx_pools["pw"].tile([128, 256], FP)
            for _ in range(12):
                nc.tensor.matmul(pwarm[:], wjunk[:, 0:128], wjunk[:, 128:384],
                                 start=True, stop=True)

            # ---- projections: kp = W2 K^T, qp = W1 Q^T (PSUM fp32) -------
            PK = ppk.tile([128, 2, 256], FP)
            for at in range(2):
                for db in range(4):
                    nc.tensor.matmul(PK[:, at, :],
                                     w2[:, db, at * 128:(at + 1) * 128],
                                     kt[:, db, :],
                                     start=(db == 0), stop=(db == 3))
            PQ = ppq.tile([128, 2, 256], FP)
            for at in range(2):
                for db in range(4):
                    nc.tensor.matmul(PQ[:, at, :],
                                     w1[:, db, at * 128:(at + 1) * 128],
                                     qt[:, db, :],
                                     start=(db == 0), stop=(db == 3))

            # ---- mode-1 factors on ACT ------------------------------------
            S1k = fk.tile([128, 2, 256], BF)
            C1k = fk.tile([128, 2, 256], BF)
            for at in range(2):
                nc.scalar.activation(S1k[:, at, :], PK[:, at, :], Act.Sin,
                                     bias=0.0, scale=THETA)
                nc.scalar.activation(C1k[:, at, :], PK[:, at, :], Act.Sin,
                                     bias=bias_hp[:, 0:1], scale=THETA)
            # FQ units: 0=s1,1=c1,2=U3=s1c1,3=U4=c1^2,4=V5=c1U3,5=V6=c1U4
            FQ = fq.tile([128, 6, 2, 256], BF)
            GQ = fq.tile([128, 6, 2, 256], BF)
            kpbf = fk.tile([128, 2, 256], BF)
            nc.scalar.activation(kpbf[:], PK[:], Act.Identity, bias=0.0)
            for at in range(2):
                nc.scalar.activation(FQ[:, 0, at, :], PQ[:, at, :], Act.Sin,
                                     bias=0.0, scale=THETA)
                nc.scalar.activation(FQ[:, 1, at, :], PQ[:, at, :], Act.Sin,
                                     bias=bias_hp[:, 0:1], scale=THETA)

            # ---- DVE: monomials, k-side mode tensors, folds ---------------
            def tt(out, a, b, op=Alu.mult):
                nc.vector.tensor_tensor(out, a, b, op=op)

            def ts(out, in0, s1, s2=None, op0=Alu.mult, op1=Alu.add):
                if s2 is None:
                    nc.vector.tensor_scalar(out, in0, float(s1), None, op0=op0)
                else:
                    nc.vector.tensor_scalar(out, in0, float(s1), float(s2),
                                            op0=op0, op1=op1)

            # k mode-2 (ready right after k sins)
            X2 = fk.tile([128, 2, 256], BF)
            Y2 = fk.tile([128, 2, 256], BF)
            tt(X2[:], S1k[:], C1k[:])
            tt(Y2[:], C1k[:], C1k[:])
            C2s = fk.tile([128, 2, 256], BF)
            S2s = fk.tile([128, 2, 256], BF)
            S2r = fk.tile([128, 2, 256], BF)
            ts(C2s[:], Y2[:], 4 * B2, -2 * B2)
            ts(S2s[:], X2[:], 4 * B2)
            ts(S2r[:], X2[:], -2 * B2 / C_LIN)
            # k mode-3 products (inputs ready early)
            T3 = fk.tile([128, 2, 256], BF)
            U3k = fk.tile([128, 2, 256], BF)
            tt(T3[:], C1k[:], X2[:])
            tt(U3k[:], C1k[:], Y2[:])
            # q monomials + folds (per-unit; group-level w3 tensors)
            tt(GQ[:, 0], FQ[:, 0], w3b1c[:])
            tt(GQ[:, 1], FQ[:, 1], w3b1c[:])
            tt(FQ[:, 2], FQ[:, 0], FQ[:, 1])
            tt(FQ[:, 3], FQ[:, 1], FQ[:, 1])
            tt(GQ[:, 2], FQ[:, 2], w3c[:])
            tt(GQ[:, 3], FQ[:, 3], w3c[:])
            tt(FQ[:, 4], FQ[:, 1], FQ[:, 2])
            tt(FQ[:, 5], FQ[:, 1], FQ[:, 3])
            tt(GQ[:, 4], FQ[:, 4], w3c[:])
            tt(GQ[:, 5], FQ[:, 5], w3c[:])
            # k mode-3 affine
            s3B = fk.tile([128, 2, 256], BF)
            c3B = fk.tile([128, 2, 256], BF)
            nc.vector.scalar_tensor_tensor(s3B[:], S1k[:], -0.25, T3[:],
                                           op0=Alu.mult, op1=Alu.add)
            nc.vector.scalar_tensor_tensor(c3B[:], C1k[:], -0.75, U3k[:],
                                           op0=Alu.mult, op1=Alu.add)
            S3s = fk.tile([128, 2, 256], BF)
            S3r = fk.tile([128, 2, 256], BF)
            C3s = fk.tile([128, 2, 256], BF)
            C3r = fk.tile([128, 2, 256], BF)
            ts(S3s[:], s3B[:], 16 * B3)
            ts(S3r[:], s3B[:], -12 * B3 / B1)
            ts(C3s[:], c3B[:], 16 * B3)
            ts(C3r[:], c3B[:], -4 * B3 / B1)

            # ---- score matmuls into two PSUM l-tiles ----------------------
            S0 = ps.tile([128, 256], FP)
            S1 = ps.tile([128, 256], FP)
            Sl = [S0, S1]
            cnt = [0, 0]
            n_mm = 20

            def score_mm(lt, lhsT, rhs):
                nc.tensor.matmul(Sl[lt][:], lhsT, rhs, start=(cnt[lt] == 0),
                                 stop=(cnt[lt] == n_mm - 1))
                cnt[lt] += 1

            # (q-unit lhsT, k rhs) pairs in readiness order
            plan = [(0, C1k), (1, S1k),            # mode 1
                    ("rep", kpbf),                 # k-linear
                    (2, C2s), (3, S2s),            # mode 2
                    ("rep", S2r),                  # mode-2 rank-1
                    (4, C3s), (5, S3s),            # mode 3
                    (0, C3r), (1, S3r)]            # mode-3 s1/c1 partners
            for u, krhs in plan:
                for at in range(2):
                    for lt in range(2):
                        lhsT = (w3rep[:, at, :] if u == "rep"
                                else GQ[:, u, at, lt * 128:(lt + 1) * 128])
                        score_mm(lt, lhsT, krhs[:, at, :])

            # ---- masked softmax over k ------------------------------------
            for lt in range(2):
                ms = tl.tile([128, 256], FP, name=f"ms{lt}")
                tt(ms[:], Sl[lt][:], maskb[:, lt, :], op=Alu.add)
                e = tl.tile([128, 256], BF, name=f"e{lt}")
                sums = tl.tile([128, 1], FP, name=f"sums{lt}")
                nc.scalar.activation(e[:], ms[:], Act.Exp, bias=0.0,
                                     accum_out=sums[:])
                recip = tl.tile([128, 1], FP, name=f"recip{lt}")
                nc.vector.reciprocal(recip[:], sums[:])
                outt = tl.tile([128, 256], FP, name=f"outt{lt}")
                nc.scalar.activation(outt[:], e[:], Act.Identity, bias=0.0,
                                     scale=recip[:, 0:1])
                nc.sync.dma_start(Od[:, lt * 256:(lt + 1) * 256], outt[:])

    nc.compile()
    return nc


def _get_nc():
    global _cached_nc
    if _cached_nc is None:
        _cached_nc = _build()
    return _cached_nc


def _pack_T(x):
    """[rows, D=512] -> bf16 [128, 4*rows] laid out as (d%128, d//128, row)."""
    xT = np.ascontiguousarray(x.T)  # [D, rows]
    r = xT.reshape(4, 128, -1).transpose(1, 0, 2)  # [128, 4, rows]
    return np.ascontiguousarray(r.reshape(128, -1).astype(ml_dtypes.bfloat16))


def _make_in_maps(inputs):
    Q = np.asarray(inputs["Q"], dtype=np.float32).reshape(B, LQ, D)
    K = np.asarray(inputs["K"], dtype=np.float32).reshape(B, LK, D)
    mask = np.asarray(inputs["mask"])
    W1 = np.asarray(inputs["W1"], dtype=np.float32)
    W2 = np.asarray(inputs["W2"], dtype=np.float32)
    w3 = np.asarray(inputs["w3"], dtype=np.float32)

    w1p = _pack_T(W1)
    w2p = _pack_T(W2)
    w3t = w3.reshape(2, 128).T.astype(np.float32)          # [128 p, 2 at]
    bc = lambda x, n: np.repeat(x[:, :, None], n,
                                axis=2).reshape(128, -1)
    w3b1c = bc(w3t * B1, 256).astype(ml_dtypes.bfloat16)    # [128, 512]
    w3c = bc(w3t, 256).astype(ml_dtypes.bfloat16)           # [128, 512]
    w3rep = bc(w3t * C_LIN, 128).astype(ml_dtypes.bfloat16)  # [128, 256]
    blob_w = np.ascontiguousarray(
        np.concatenate([w3b1c, w3c, w3rep], axis=1))        # [128, 1280]

    maps = []
    for c in range(N_CORES):
        blob_k = np.concatenate([_pack_T(K[c]), w2p], axis=1)
        blob_q = np.concatenate([_pack_T(Q[c]), w1p], axis=1)
        mb = np.where(mask[c] == 0, -100.0, 0.0).astype(ml_dtypes.bfloat16)
        mb = np.ascontiguousarray(
            mb.reshape(2, 128, 256).transpose(1, 0, 2).reshape(128, 512))
        maps.append(dict(blob_k=np.ascontiguousarray(blob_k),
                         blob_q=np.ascontiguousarray(blob_q),
                         blob_w=blob_w,
                         maskb=mb))
    return maps


def _run(inputs, trace=False, tmpdir=None):
    from concourse.bass_utils import run_bass_kernel_spmd

    nc = _get_nc()
    in_maps = _make_in_maps(inputs)
    res = run_bass_kernel_spmd(
        nc, in_maps, list(range(N_CORES)), trace=trace, tmpdir=tmpdir
    )
    out = np.empty((B, LQ, LK), np.float32)
    for c in range(N_CORES):
        o = np.asarray(res.results[c]["out"], dtype=np.float32)  # [128, 512]
        out[c] = o.reshape(128, 2, 256).transpose(1, 0, 2).reshape(256, 256)
    return out, res


def kernel(**inputs) -> np.ndarray:
    out, _ = _run(inputs, trace=False)
    return out


# revision 8
# speedup vs baseline: 1.9248x; 1.0367x over previous
"""Additive (Bahdanau) attention for Trainium2, SPMD over 8 NeuronCores.

score[b,l,k] = sum_a w3[a] * tanh(qp[b,l,a] + kp[b,k,a]);  masked softmax over k
  qp = Q @ W1^T, kp = K @ W2^T

Sharding: data-parallel over batch B=8 (one batch per core), weights replicated.

Algorithm: 3-mode harmonic sine-ridge fit of tanh, theta tuned on the
empirical z = qp+kp distribution and validated against the measured ACT-Sin
error curve:

  tanh(z) ~= c_lin*z + b1 sin(t z) + b2 sin(2 t z) + b3 sin(3 t z)

Each sine mode splits by angle addition into separable products over the
a-axis -> bf16 tensor-engine matmuls with contraction A. Mode-1 sin/cos come
straight from ACT Sin reading the projection PSUM. Higher modes avoid the
slow 1x scalar_tensor_tensor path by expanding the q-side in *monomials* of
mode-1 factors (U3=s1c1, U4=c1^2, V5=c1U3, V6=c1U4 -- pure 2x tensor_tensor)
and keeping the few affine fix-ups on the k side:

  sin2(q+k): 2 U3q c2k + 2 U4q s2k - s2k        (rank-1 in k -> extra matmul)
  sin3(q+k): (4V5-s1)q c3k + (4V6-3c1)q s3k     (s1/c1 lhsT reused)

w3 folds onto the q side as plain tensor_tensor against host-replicated
broadcast tensors (AP-scalar tensor_scalar is 10x slow). The q-side linear
rank-1 term cancels in the softmax; the k-side linear term is one matmul
against a host-built w3*c_lin lhsT. Softmax: additive -100 mask bias, ACT Exp
with accumulated row sums, normalize via ACT Identity with per-partition
reciprocal scale.
"""

import sys

import numpy as np

if "/opt/trn_rl_repo" not in sys.path:
    sys.path.insert(0, "/opt/trn_rl_repo")

import ml_dtypes

B, LQ, LK, D, A = 8, 256, 256, 512, 256
N_CORES = 8

THETA = 0.78
C_LIN = 0.2467
B1, B2, B3 = 0.50004, 0.12708, 0.04244

_cached_nc = None


def _build():
    from contextlib import ExitStack

    import concourse.mybir as mybir
    from concourse import tile
    from concourse.bacc import Bacc

    FP = mybir.dt.float32
    BF = mybir.dt.bfloat16
    Act = mybir.ActivationFunctionType
    Alu = mybir.AluOpType

    nc = Bacc()
    KBd = nc.declare_dram_parameter("blob_k", [128, 2048], BF, isOutput=False)
    QBd = nc.declare_dram_parameter("blob_q", [128, 2048], BF, isOutput=False)
    # w3 blob: w3*b1 bcast [2,256] | w3 bcast [2,256] | w3*c_lin rep [2,128]
    WBd = nc.declare_dram_parameter("blob_w", [128, 1280], BF, isOutput=False)
    Md = nc.declare_dram_parameter("maskb", [128, 512], BF, isOutput=False)
    Od = nc.declare_dram_parameter("out", [128, 512], BF, isOutput=True)

    with tile.TileContext(nc) as tc:
        with ExitStack() as ctx:
            const = ctx.enter_context(tc.tile_pool(name="const", bufs=1))
            inp = ctx.enter_context(tc.tile_pool(name="inp", bufs=1))
            fk = ctx.enter_context(tc.tile_pool(name="fk", bufs=1))
            fq = ctx.enter_context(tc.tile_pool(name="fq", bufs=1))
            tl = ctx.enter_context(tc.tile_pool(name="tl", bufs=1))
            pw = ctx.enter_context(tc.tile_pool(name="pw", bufs=1, space="PSUM"))
            ctx_pools = {"pw": pw}
            ppk = ctx.enter_context(tc.tile_pool(name="ppk", bufs=1, space="PSUM"))
            ppq = ctx.enter_context(tc.tile_pool(name="ppq", bufs=1, space="PSUM"))
            ps = ctx.enter_context(tc.tile_pool(name="ps", bufs=1, space="PSUM"))

            # ---- input DMAs (HWDGE; complete during the startup barrier) --
            kin = inp.tile([128, 2, 4, 256], BF)     # [kt | w2]
            nc.sync.dma_start(kin[:], KBd.rearrange("p (i db x) -> p i db x",
                                                    i=2, db=4))
            qin = inp.tile([128, 2, 4, 256], BF)     # [qt | w1]
            nc.sync.dma_start(qin[:], QBd.rearrange("p (i db x) -> p i db x",
                                                    i=2, db=4))
            win = inp.tile([128, 1280], BF)
            nc.sync.dma_start(win[:], WBd[:])
            maskb = inp.tile([128, 2, 256], BF)
            nc.sync.dma_start(maskb[:], Md.rearrange("p (i k) -> p i k", i=2))

            kt, w2 = kin[:, 0], kin[:, 1]
            qt, w1 = qin[:, 0], qin[:, 1]
            w3b1c = win[:, 0:512].rearrange("p (i k) -> p i k", i=2)
            w3c = win[:, 512:1024].rearrange("p (i k) -> p i k", i=2)
            w3rep = win[:, 1024:1280].rearrange("p (i k) -> p i k", i=2)

            junk = const.tile([128, 8], BF)
            nc.gpsimd.memset(junk[:], 0.125)
            bias_hp = const.tile([128, 1], FP)
            nc.gpsimd.memset(bias_hp[:], float(np.pi / 2))
            dummy = const.tile([128, 8], FP)
            # first ACT op: forces the sin table load at t~0
            nc.scalar.activation(dummy[:], junk[:], Act.Sin, bias=0.0)
            wjunk = const.tile([128, 384], BF)
            nc.vector.memset(wjunk[:], 0.125)
            pwarm = ctx_pools["pw"].tile([128, 256], FP)
            for _ in range(12):
                nc.tensor.matmul(pwarm[:], wjunk[:, 0:128], wjunk[:, 128:384],
                                 start=True, stop=True)

            # ---- projections: kp = W2 K^T, qp = W1 Q^T (PSUM fp32) -------
            PK = [ppk.tile([128, 256], FP, name=f"pk{at}") for at in range(2)]
            for at in range(2):
                for db in range(4):
                    nc.tensor.matmul(PK[at][:],
                                     w2[:, db, at * 128:(at + 1) * 128],
                                     kt[:, db, :],
                                     start=(db == 0), stop=(db == 3))
            PQ = [ppq.tile([128, 256], FP, name=f"pq{at}") for at in range(2)]
            for at in range(2):
                for db in range(4):
                    nc.tensor.matmul(PQ[at][:],
                                     w1[:, db, at * 128:(at + 1) * 128],
                                     qt[:, db, :],
                                     start=(db == 0), stop=(db == 3))

            for _ in range(6):
                nc.tensor.matmul(pwarm[:], wjunk[:, 0:128], wjunk[:, 128:384],
                                 start=True, stop=True)

            # ---- mode-1 factors on ACT ------------------------------------
            S1k = fk.tile([128, 2, 256], BF)
            C1k = fk.tile([128, 2, 256], BF)
            for at in range(2):
                nc.scalar.activation(S1k[:, at, :], PK[at][:], Act.Sin,
                                     bias=0.0, scale=THETA)
                nc.scalar.activation(C1k[:, at, :], PK[at][:], Act.Sin,
                                     bias=bias_hp[:, 0:1], scale=THETA)
            # FQ units: 0=s1,1=c1,2=U3=s1c1,3=U4=c1^2,4=V5=c1U3,5=V6=c1U4
            FQ = fq.tile([128, 6, 2, 256], BF)
            GQ = fq.tile([128, 6, 2, 256], BF)
            kpbf = fk.tile([128, 2, 256], BF)
            for at in range(2):
                nc.scalar.activation(FQ[:, 0, at, :], PQ[at][:], Act.Sin,
                                     bias=0.0, scale=THETA)
                nc.scalar.activation(FQ[:, 1, at, :], PQ[at][:], Act.Sin,
                                     bias=bias_hp[:, 0:1], scale=THETA)
            for at in range(2):
                nc.scalar.activation(kpbf[:, at, :], PK[at][:], Act.Identity,
                                     bias=0.0)

            # ---- DVE: monomials, k-side mode tensors, folds ---------------
            def tt(out, a, b, op=Alu.mult):
                nc.vector.tensor_tensor(out, a, b, op=op)

            def ts(out, in0, s1, s2=None, op0=Alu.mult, op1=Alu.add):
                if s2 is None:
                    nc.vector.tensor_scalar(out, in0, float(s1), None, op0=op0)
                else:
                    nc.vector.tensor_scalar(out, in0, float(s1), float(s2),
                                            op0=op0, op1=op1)

            # k mode-2 (ready right after k sins)
            X2 = fk.tile([128, 2, 256], BF)
            Y2 = fk.tile([128, 2, 256], BF)
            tt(X2[:], S1k[:], C1k[:])
            tt(Y2[:], C1k[:], C1k[:])
            C2s = fk.tile([128, 2, 256], BF)
            S2s = fk.tile([128, 2, 256], BF)
            S2r = fk.tile([128, 2, 256], BF)
            ts(C2s[:], Y2[:], 4 * B2, -2 * B2)
            ts(S2s[:], X2[:], 4 * B2)
            ts(S2r[:], X2[:], -2 * B2 / C_LIN)
            # k mode-3 products (inputs ready early)
            T3 = fk.tile([128, 2, 256], BF)
            U3k = fk.tile([128, 2, 256], BF)
            tt(T3[:], C1k[:], X2[:])
            tt(U3k[:], C1k[:], Y2[:])
            # q monomials + folds (per-unit; group-level w3 tensors)
            tt(GQ[:, 0], FQ[:, 0], w3b1c[:])
            tt(GQ[:, 1], FQ[:, 1], w3b1c[:])
            tt(FQ[:, 2], FQ[:, 0], FQ[:, 1])
            tt(FQ[:, 3], FQ[:, 1], FQ[:, 1])
            tt(GQ[:, 2], FQ[:, 2], w3c[:])
            tt(GQ[:, 3], FQ[:, 3], w3c[:])
            tt(FQ[:, 4], FQ[:, 1], FQ[:, 2])
            tt(FQ[:, 5], FQ[:, 1], FQ[:, 3])
            tt(GQ[:, 4], FQ[:, 4], w3c[:])
            tt(GQ[:, 5], FQ[:, 5], w3c[:])
            # k mode-3 affine
            s3B = fk.tile([128, 2, 256], BF)
            c3B = fk.tile([128, 2, 256], BF)
            nc.vector.scalar_tensor_tensor(s3B[:], S1k[:], -0.25, T3[:],
                                           op0=Alu.mult, op1=Alu.add)
            nc.vector.scalar_tensor_tensor(c3B[:], C1k[:], -0.75, U3k[:],
                                           op0=Alu.mult, op1=Alu.add)
            S3s = fk.tile([128, 2, 256], BF)
            S3r = fk.tile([128, 2, 256], BF)
            C3s = fk.tile([128, 2, 256], BF)
            C3r = fk.tile([128, 2, 256], BF)
            ts(S3s[:], s3B[:], 16 * B3)
            ts(S3r[:], s3B[:], -12 * B3 / B1)
            ts(C3s[:], c3B[:], 16 * B3)
            ts(C3r[:], c3B[:], -4 * B3 / B1)

            # ---- score matmuls into two PSUM l-tiles ----------------------
            S0 = ps.tile([128, 256], FP)
            S1 = ps.tile([128, 256], FP)
            Sl = [S0, S1]
            cnt = [0, 0]
            n_mm = 20

            def score_mm(lt, lhsT, rhs):
                nc.tensor.matmul(Sl[lt][:], lhsT, rhs, start=(cnt[lt] == 0),
                                 stop=(cnt[lt] == n_mm - 1))
                cnt[lt] += 1

            # (q-unit lhsT, k rhs) pairs in readiness order; modes 1-2
            # interleave l-tiles, mode 3 closes lt0 first so its softmax
            # tail overlaps lt1's remaining matmuls
            early = [(0, C1k), (1, S1k),           # mode 1
                     ("rep", kpbf),                # k-linear
                     (2, C2s), (3, S2s),           # mode 2
                     ("rep", S2r)]                 # mode-2 rank-1
            late = [(4, C3s), (5, S3s),            # mode 3
                    (0, C3r), (1, S3r)]            # mode-3 s1/c1 partners

            def mm_of(lt, u, krhs, at):
                lhsT = (w3rep[:, at, :] if u == "rep"
                        else GQ[:, u, at, lt * 128:(lt + 1) * 128])
                score_mm(lt, lhsT, krhs[:, at, :])

            for u, krhs in early:
                for at in range(2):
                    for lt in range(2):
                        mm_of(lt, u, krhs, at)
            for lt in range(2):
                for u, krhs in late:
                    for at in range(2):
                        mm_of(lt, u, krhs, at)

            # ---- masked softmax over k ------------------------------------
            for lt in range(2):
                ms = tl.tile([128, 256], FP, name=f"ms{lt}")
                tt(ms[:], Sl[lt][:], maskb[:, lt, :], op=Alu.add)
                e = tl.tile([128, 256], BF, name=f"e{lt}")
                sums = tl.tile([128, 1], FP, name=f"sums{lt}")
                nc.scalar.activation(e[:], ms[:], Act.Exp, bias=0.0,
                                     accum_out=sums[:])
                recip = tl.tile([128, 1], FP, name=f"recip{lt}")
                nc.vector.reciprocal(recip[:], sums[:])
                outt = tl.tile([128, 256], BF, name=f"outt{lt}")
                nc.scalar.activation(outt[:], e[:], Act.Identity, bias=0.0,
                                     scale=recip[:, 0:1])
                eng = nc.scalar if lt == 0 else nc.sync
                eng.dma_start(Od[:, lt * 256:(lt + 1) * 256], outt[:])

    nc.compile()
    return nc


def _get_nc():
    global _cached_nc
    if _cached_nc is None:
        _cached_nc = _build()
    return _cached_nc


def _pack_T(x):
    """[rows, D=512] -> bf16 [128, 4*rows] laid out as (d%128, d//128, row)."""
    xT = np.ascontiguousarray(x.T)  # [D, rows]
    r = xT.reshape(4, 128, -1).transpose(1, 0, 2)  # [128, 4, rows]
    return np.ascontiguousarray(r.reshape(128, -1).astype(ml_dtypes.bfloat16))


def _make_in_maps(inputs):
    Q = np.asarray(inputs["Q"], dtype=np.float32).reshape(B, LQ, D)
    K = np.asarray(inputs["K"], dtype=np.float32).reshape(B, LK, D)
    mask = np.asarray(inputs["mask"])
    W1 = np.asarray(inputs["W1"], dtype=np.float32)
    W2 = np.asarray(inputs["W2"], dtype=np.float32)
    w3 = np.asarray(inputs["w3"], dtype=np.float32)

    w1p = _pack_T(W1)
    w2p = _pack_T(W2)
    w3t = w3.reshape(2, 128).T.astype(np.float32)          # [128 p, 2 at]
    bc = lambda x, n: np.repeat(x[:, :, None], n,
                                axis=2).reshape(128, -1)
    w3b1c = bc(w3t * B1, 256).astype(ml_dtypes.bfloat16)    # [128, 512]
    w3c = bc(w3t, 256).astype(ml_dtypes.bfloat16)           # [128, 512]
    w3rep = bc(w3t * C_LIN, 128).astype(ml_dtypes.bfloat16)  # [128, 256]
    blob_w = np.ascontiguousarray(
        np.concatenate([w3b1c, w3c, w3rep], axis=1))        # [128, 1280]

    maps = []
    for c in range(N_CORES):
        blob_k = np.concatenate([_pack_T(K[c]), w2p], axis=1)
        blob_q = np.concatenate([_pack_T(Q[c]), w1p], axis=1)
        mb = np.where(mask[c] == 0, -100.0, 0.0).astype(ml_dtypes.bfloat16)
        mb = np.ascontiguousarray(
            mb.reshape(2, 128, 256).transpose(1, 0, 2).reshape(128, 512))
        maps.append(dict(blob_k=np.ascontiguousarray(blob_k),
                         blob_q=np.ascontiguousarray(blob_q),
                         blob_w=blob_w,
                         maskb=mb))
    return maps


def _run(inputs, trace=False, tmpdir=None):
    from concourse.bass_utils import run_bass_kernel_spmd

    nc = _get_nc()
    in_maps = _make_in_maps(inputs)
    res = run_bass_kernel_spmd(
        nc, in_maps, list(range(N_CORES)), trace=trace, tmpdir=tmpdir
    )
    out = np.empty((B, LQ, LK), np.float32)
    for c in range(N_CORES):
        o = np.asarray(res.results[c]["out"], dtype=np.float32)  # [128, 512]
        out[c] = o.reshape(128, 2, 256).transpose(1, 0, 2).reshape(256, 256)
    return out, res


def kernel(**inputs) -> np.ndarray:
    out, _ = _run(inputs, trace=False)
    return out
